# revision 3
# baseline (speedup 1.0000x reference)
"""Trainium2 Bass kernel for nn_CapChMatch (capsule channel-routing).

Math (reference):
  g[b0,b1,c,k,p] = xpad[b0,b1,c, indexm[k*P+p]]          (im2col gather)
  u_hat[(b1,k),(b0,c,p),s] = g * W[c,k,s]
  3 rounds of dynamic routing with softmax over s=8 and squash over the
  n2 = (b0,c,p) = 131072-element reduction axis; output (36,1,8).

Distribution: shard by n1 = (b1,k) rows (36 rows -> 8 cores, 5/4 each with a
padded duplicate slot on 4-row cores). Each core touches exactly one b1 slice
of x and computes its output rows fully independently - no collectives.

Per-core device layout: partitions = (b0,c) = 128, free = p (1024 per k slot).
 - gather: gpsimd ap_gather (shared index list per 16-partition group)
 - exp(v_s*W_s*g): ScalarE activation with per-partition scale
 - softmax-weighted reduction sum_p W_s*g*e_s/D: one scalar_tensor_tensor
   per plane with accum_out (fused multiply+reduce on VectorE)
 - cross-partition sums + broadcasts: TensorE matmuls with ones vectors
 - squash via Ln/Exp (one ACT table set); reciprocal_approx_fast for 1/D
"""
import os
import sys

import numpy as np

for _p in ("/opt/trn_rl_repo", "/root/.axon_site/_ro/trn_rl_repo"):
    if os.path.isdir(_p) and _p not in sys.path:
        sys.path.insert(0, _p)

import concourse.bacc as bacc
import concourse.tile as tile
from concourse import library_config, mybir
from concourse.bass_utils import run_bass_kernel_spmd

f32 = mybir.dt.float32
i16 = mybir.dt.int16
ALU = mybir.AluOpType
ACTF = mybir.ActivationFunctionType
AXL = mybir.AxisListType

B0, B1, C, H, W = 2, 4, 64, 32, 32
KLEN, S, P = 9, 8, 1024
NK = 5          # k-slots per core (4-row cores duplicate one slot)
NCOL = NK * S   # 40
ROUTINGS = 3
USE_POOL = os.environ.get("KERNEL_USE_POOL", "1") == "1"

_PROGRAM_CACHE = {}


def _build_program(npix):
    nc = bacc.Bacc("TRN2", target_bir_lowering=False, debug=False)
    xf_d = nc.dram_tensor("xf", [128, npix], f32, kind="ExternalInput").ap()
    idx_d = nc.dram_tensor("idx", [128, NK * P // 16], i16,
                           kind="ExternalInput").ap()
    w_d = nc.dram_tensor("wcols", [128, NCOL], f32, kind="ExternalInput").ap()
    out_d = nc.dram_tensor("out", [1, NCOL], f32, kind="ExternalOutput").ap()

    with tile.TileContext(nc) as tc:
        with tc.tile_pool(name="const", bufs=1) as const, \
             tc.tile_pool(name="epool", bufs=2) as epool, \
             tc.tile_pool(name="work", bufs=2) as work, \
             tc.tile_pool(name="small", bufs=3) as small, \
             tc.tile_pool(name="psum", bufs=2, space="PSUM") as psum:

            xf_sb = const.tile([128, npix], f32)
            nc.sync.dma_start(xf_sb[:], xf_d)
            idx_sb = const.tile([128, NK * P // 16], i16)
            nc.sync.dma_start(idx_sb[:], idx_d)
            w_sb = const.tile([128, NCOL], f32)
            nc.sync.dma_start(w_sb[:], w_d)

            ones_col = const.tile([128, 1], f32)
            nc.vector.memset(ones_col[:], 1.0)
            ones_row = const.tile([1, 128], f32)
            nc.vector.memset(ones_row[:], 1.0)
            eps_t = const.tile([128, 1], f32)
            nc.vector.memset(eps_t[:], 1e-8)

            g_all = const.tile([128, NK * P], f32)
            nc.gpsimd.load_library(library_config.ap_gather)
            nc.gpsimd.ap_gather(g_all[:], xf_sb[:], idx_sb[:], channels=128,
                                num_elems=npix, d=1, num_idxs=NK * P)
            if USE_POOL:
                nc.gpsimd.load_library(library_config.standard)

            def finisher(acols, scale):
                """(128,NCOL) per-partition partials -> broadcast col sums,
                scaled: T[q, j] = scale * sum_part acols[part, j]."""
                p1 = psum.tile([1, NCOL], f32, tag="p1")
                nc.tensor.matmul(p1[:], ones_col[:], acols[:], start=True,
                                 stop=True)
                s1 = small.tile([1, NCOL], f32, tag="s1")
                nc.scalar.copy(s1[:], p1[:])
                pbc = psum.tile([128, NCOL], f32, tag="pbc")
                nc.tensor.matmul(pbc[:], ones_row[:], s1[:], start=True,
                                 stop=True)
                t_all = small.tile([128, NCOL], f32, tag="T")
                nc.scalar.activation(t_all[:], pbc[:], ACTF.Copy, scale=scale)
                return t_all

            def squash_scale(t_all):
                """t(128,NK): per-slot squash scale n2/((1+n2)*sqrt(n2+eps)),
                n2 = sum_s T^2."""
                sq = small.tile([128, NCOL], f32, tag="sq")
                nc.vector.tensor_tensor(sq[:], t_all[:], t_all[:], ALU.mult)
                n2 = small.tile([128, NK], f32, tag="n2")
                nc.vector.tensor_reduce(
                    out=n2[:].rearrange("q (a b) -> q a b", b=1),
                    in_=sq[:].rearrange("q (a b) -> q a b", a=NK),
                    axis=AXL.X, op=ALU.add)
                ln_t = small.tile([128, NK], f32, tag="ln")
                nc.scalar.activation(ln_t[:], n2[:], ACTF.Ln, bias=eps_t[:])
                rsq = small.tile([128, NK], f32, tag="rsq")
                nc.scalar.activation(rsq[:], ln_t[:], ACTF.Exp, scale=-0.5)
                b1p = small.tile([128, NK], f32, tag="b1p")
                nc.vector.tensor_scalar(out=b1p[:], in0=n2[:], scalar1=1.0,
                                        scalar2=None, op0=ALU.add)
                rb = small.tile([128, NK], f32, tag="rb")
                nc.vector.reciprocal(out=rb[:], in_=b1p[:])
                t0 = small.tile([128, NK], f32, tag="t0")
                nc.vector.tensor_tensor(t0[:], n2[:], rb[:], ALU.mult)
                tsc = small.tile([128, NK], f32, tag="tsc")
                nc.vector.tensor_tensor(tsc[:], t0[:], rsq[:], ALU.mult)
                return tsc

            def squash_to_wvp(t_all):
                """wvp (128,NCOL): col ki*8+s = W[c,k,s]*v_s - W[c,k,0]*v_0."""
                tsc = squash_scale(t_all)
                wv = small.tile([128, NCOL], f32, tag="wv")
                for ki in range(NK):
                    cs = slice(ki * S, (ki + 1) * S)
                    nc.vector.scalar_tensor_tensor(
                        out=wv[:, cs], in0=t_all[:, cs],
                        scalar=tsc[:, ki:ki + 1], in1=w_sb[:, cs],
                        op0=ALU.mult, op1=ALU.mult)
                wvp = small.tile([128, NCOL], f32, tag="wvp")
                for ki in range(NK):
                    cs = slice(ki * S, (ki + 1) * S)
                    nc.vector.tensor_scalar(
                        out=wvp[:, cs], in0=wv[:, cs],
                        scalar1=wv[:, ki * S:ki * S + 1], scalar2=None,
                        op0=ALU.subtract)
                return wvp

            # ---- routing iteration 1: c uniform = 1/8 -> plain reductions
            # Gsum[(b0,c)] = sum_p g ; acols[:, ki*8+s] = W[c,k,s] * Gsum
            acols = small.tile([128, NCOL], f32, tag="acols")
            gsum = small.tile([128, NK], f32, tag="gsum")
            for ki in range(NK):
                scr1 = work.tile([128, P], f32, tag="scr")
                nc.vector.tensor_scalar(
                    out=scr1[:], in0=g_all[:, ki * P:(ki + 1) * P],
                    scalar1=1.0, scalar2=0.0, op0=ALU.mult, op1=ALU.add,
                    accum_out=gsum[:, ki:ki + 1])
            for ki in range(NK):
                nc.vector.tensor_scalar(
                    out=acols[:, ki * S:(ki + 1) * S],
                    in0=w_sb[:, ki * S:(ki + 1) * S],
                    scalar1=gsum[:, ki:ki + 1], scalar2=None, op0=ALU.mult)
            t_all = finisher(acols, 1.0 / S)
            wvp = squash_to_wvp(t_all)

            # ---- routing iterations 2..ROUTINGS: softmax-weighted reductions
            for it in range(1, ROUTINGS):
                acols = small.tile([128, NCOL], f32, tag="acols")
                for ki in range(NK):
                    g_ki = g_all[:, ki * P:(ki + 1) * P]
                    e_t = epool.tile([128, S - 1, P], f32, tag="e")
                    for s in range(1, S):
                        nc.scalar.activation(
                            e_t[:, s - 1, :], g_ki, ACTF.Exp,
                            scale=wvp[:, ki * S + s:ki * S + s + 1])
                    # denominator D = 1 + sum_s e_s (e_0 == 1 by the shift)
                    dv = work.tile([128, P], f32, tag="dv")
                    nc.vector.tensor_tensor(dv[:], e_t[:, 0, :], e_t[:, 1, :],
                                            ALU.add)
                    nc.vector.tensor_tensor(dv[:], dv[:], e_t[:, 2, :],
                                            ALU.add)
                    dd = work.tile([128, P], f32, tag="dd")
                    pooleng = nc.gpsimd if USE_POOL else nc.vector
                    dp = work.tile([128, P], f32, tag="dp")
                    pooleng.tensor_tensor(dp[:], e_t[:, 3, :], e_t[:, 4, :],
                                          ALU.add)
                    dp2 = work.tile([128, P], f32, tag="dp2")
                    pooleng.tensor_tensor(dp2[:], e_t[:, 5, :], e_t[:, 6, :],
                                          ALU.add)
                    dp3 = work.tile([128, P], f32, tag="dp3")
                    pooleng.tensor_tensor(dp3[:], dp[:], dp2[:], ALU.add)
                    nc.vector.scalar_tensor_tensor(
                        out=dd[:], in0=dv[:], scalar=1.0, in1=dp3[:],
                        op0=ALU.add, op1=ALU.add)
                    rr = work.tile([128, P], f32, tag="rr")
                    nc.vector.reciprocal_approx_fast(rr[:], dd[:])
                    gr = work.tile([128, P], f32, tag="gr")
                    nc.vector.tensor_tensor(gr[:], g_ki, rr[:], ALU.mult)
                    scratch = work.tile([128, P], f32, tag="scr")
                    nc.vector.tensor_scalar(
                        out=scratch[:], in0=gr[:],
                        scalar1=w_sb[:, ki * S:ki * S + 1], scalar2=0.0,
                        op0=ALU.mult, op1=ALU.add,
                        accum_out=acols[:, ki * S:ki * S + 1])
                    for s in range(1, S):
                        nc.vector.scalar_tensor_tensor(
                            out=scratch[:], in0=e_t[:, s - 1, :],
                            scalar=w_sb[:, ki * S + s:ki * S + s + 1],
                            in1=gr[:], op0=ALU.mult, op1=ALU.mult,
                            accum_out=acols[:, ki * S + s:ki * S + s + 1])
                t_all = finisher(acols, 1.0)
                if it < ROUTINGS - 1:
                    wvp = squash_to_wvp(t_all)
                else:
                    tsc = squash_scale(t_all)
                    vout = small.tile([128, NCOL], f32, tag="vout")
                    for ki in range(NK):
                        cs = slice(ki * S, (ki + 1) * S)
                        nc.vector.tensor_scalar(
                            out=vout[:, cs], in0=t_all[:, cs],
                            scalar1=tsc[:, ki:ki + 1], scalar2=None,
                            op0=ALU.mult)
                    out01 = small.tile([128, NCOL], f32, tag="out01")
                    nc.vector.tensor_scalar(out=out01[:], in0=vout[:],
                                            scalar1=0.5, scalar2=0.5,
                                            op0=ALU.mult, op1=ALU.add)
                    nc.sync.dma_start(out_d, out01[0:1, :])
    nc.compile()
    return nc


def _core_k_lists():
    """core -> (b1, [k slots]) ; odd cores pad with a duplicate k."""
    lists = []
    for core in range(8):
        b1 = core // 2
        ks = [0, 1, 2, 3, 4] if core % 2 == 0 else [5, 6, 7, 8, 8]
        lists.append((b1, ks))
    return lists


def kernel(x, weight, indexm, padding):
    x = np.asarray(x, dtype=np.float32)
    weight = np.asarray(weight, dtype=np.float32)
    indexm = np.asarray(indexm)
    p = int(np.asarray(padding))
    b0, b1n, c, h, w = x.shape
    assert (b0, b1n, c, h, w) == (B0, B1, C, H, W), x.shape
    hp, wp = h + 2 * p, w + 2 * p
    npix = hp * wp

    xpad = np.pad(x, ((0, 0), (0, 0), (0, 0), (p, p), (p, p)))
    xflat = xpad.reshape(B0, B1, C, npix)
    idx_clip = np.clip(indexm.astype(np.int64), 0, npix - 1).reshape(KLEN, P)
    w_all = weight[0, 0, :, :, 0, :]          # (C, KLEN, S)

    in_maps = []
    for core, (b1i, ks) in enumerate(_core_k_lists()):
        xf_core = np.ascontiguousarray(
            xflat[:, b1i].reshape(128, npix), dtype=np.float32)
        idxc = idx_clip[ks].ravel().astype(np.int16)          # (NK*P,)
        blk = idxc.reshape(NK * P // 16, 16).T                # (16, NK*P/16)
        idx_wrapped = np.tile(blk, (8, 1)).astype(np.int16)   # (128, ...)
        wc = w_all[:, ks, :].reshape(C, NCOL)                 # (64, 40)
        wcols = np.tile(wc, (B0, 1)).astype(np.float32)       # (128, 40)
        in_maps.append({"xf": xf_core, "idx": idx_wrapped, "wcols": wcols})

    if npix not in _PROGRAM_CACHE:
        _PROGRAM_CACHE[npix] = _build_program(npix)
    nc = _PROGRAM_CACHE[npix]

    res = run_bass_kernel_spmd(nc, in_maps, core_ids=list(range(8)))

    out_full = np.zeros((B1 * KLEN, 1, S), dtype=np.float32)
    for core, (b1i, ks) in enumerate(_core_k_lists()):
        rows = res.results[core]["out"].reshape(NK, S)
        nreal = 5 if core % 2 == 0 else 4
        for ki in range(nreal):
            out_full[b1i * KLEN + ks[ki], 0, :] = rows[ki]
    return out_full
